# revision 1
# baseline (speedup 1.0000x reference)
"""Trainium2 Bass kernel for CalculateSLayer GNN message passing.

Computes, for adj [L, L, 2] f32 and h [L, D] f32 with A = adj.sum(-1):
    h_in[j, d]  = sum_i A[i, j] * h[i, d]   (= A.T @ h)
    h_out[i, d] = sum_j A[i, j] * h[j, d]   (= A @ h)

Sharding: rows of A across 8 NeuronCores. Core m holds A[m*512:(m+1)*512, :]:
  - h_out rows are fully local:      h_out_blk = A_blk @ h
  - h_in is a partial sum per core:  p_in      = A_blk.T @ h_blk
    (the 8 partials are summed on the host during unshard)

On-chip per core: DMA the adj row block in 512-wide j-windows, edge-sum
to bf16 A tiles on VectorE, PE-transpose A tiles for the j-contraction
(h_out) GEMM, and run both GEMMs as [d, *] outputs with h-slices as the
stationary operand (N=512 moving streams, fp32 PSUM accumulation).
ScalarE evicts all PSUM results; outputs are written transposed in bf16
and assembled/summed on the host.
"""

import numpy as np

L = 4096
D = 150
NCORES = 8
R = L // NCORES  # 512 rows per core
P = 128  # partitions
IC = R // P  # 4 i-chunks per core
JW = 512  # j-window width
NW = L // JW  # 8 windows
JCW = JW // P  # 4 j-chunks per window
NJC = L // P  # 32 j-chunks total

_NC_CACHE = {}
LAST_RESULTS = None


def _ensure_ntff_hook():
    """Register the axon NTFF profile hook if the image's antenv lacks it.

    The boot shim (trn_agent_boot.trn_boot) only registers the hook when
    ``antenv.axon_hooks`` is importable; on images where it isn't, tracing
    raises ModuleNotFoundError. Inject an equivalent in-memory module and
    register the ctypes-based hook against libaxon_pjrt.so.
    """
    import sys
    import types

    try:
        from antenv.axon_hooks import get_axon_ntff_profile_hook  # noqa: F401

        return
    except ImportError:
        pass

    mod = types.ModuleType("antenv.axon_hooks")
    _state = {"hook": None}
    mod.set_axon_ntff_profile_hook = lambda h: _state.__setitem__("hook", h)
    mod.get_axon_ntff_profile_hook = lambda: _state["hook"]
    sys.modules["antenv.axon_hooks"] = mod
    import antenv

    antenv.axon_hooks = mod

    so_path = "/opt/axon/libaxon_pjrt.so"
    try:
        from trn_agent_boot.trn_boot import _ntff_profile_via_ctypes

        hook = _ntff_profile_via_ctypes(so_path)
        if hook is not None:
            mod.set_axon_ntff_profile_hook(hook)
    except Exception:
        pass

    # artifact upload has no bucket in this container; make it a no-op
    try:
        from concourse import bass_utils

        bass_utils.upload_artifacts = lambda tmpdir: tmpdir
    except Exception:
        pass


def _build_nc():
    import concourse.bacc as bacc
    import concourse.tile as tile
    import concourse.mybir as mybir
    from concourse.masks import make_identity

    f32 = mybir.dt.float32
    bf16 = mybir.dt.bfloat16

    nc = bacc.Bacc(
        "TRN2", target_bir_lowering=False, debug=False, num_devices=NCORES
    )
    adj_d = nc.dram_tensor("adj_blk", [R, L, 2], f32, kind="ExternalInput").ap()
    h_d = nc.dram_tensor("h", [L, D], f32, kind="ExternalInput").ap()
    hb_d = nc.dram_tensor("h_blk", [R, D], f32, kind="ExternalInput").ap()
    # outputs are produced transposed: [D, ...]; the host transposes back
    pin_d = nc.dram_tensor("p_inT", [D, L], bf16, kind="ExternalOutput").ap()
    hout_d = nc.dram_tensor("h_outT_blk", [D, R], bf16, kind="ExternalOutput").ap()

    DT = ((0, 128), (128, D))  # d-tile splits (M <= 128)

    with tile.TileContext(nc) as tc:
        with (
            tc.tile_pool(name="const", bufs=1) as const_pool,
            tc.tile_pool(name="adj", bufs=4) as adj_pool,
            tc.tile_pool(name="abf", bufs=3) as abf_pool,
            tc.tile_pool(name="at", bufs=3) as at_pool,
            tc.tile_pool(name="pouts", bufs=3) as pout_pool,
            tc.tile_pool(name="pinps", bufs=2, space="PSUM") as pin_psum,
            tc.tile_pool(name="atps", bufs=2, space="PSUM") as at_psum,
            tc.tile_pool(name="houtps", bufs=1, space="PSUM") as hout_psum,
        ):
            ident = const_pool.tile([P, P], bf16)
            make_identity(nc, ident[:])

            # full h, laid out [p, chunk, d] with j = chunk*128 + p
            h_sb = const_pool.tile([P, NJC, D], f32)
            nc.scalar.dma_start(h_sb[:], h_d.rearrange("(c p) d -> p c d", p=P))
            h_bf = const_pool.tile([P, NJC, D], bf16)
            nc.vector.tensor_copy(h_bf[:], h_sb[:])

            # this core's row block of h, [p, ic, d] with i = ic*128 + p
            hb_sb = const_pool.tile([P, IC, D], f32)
            nc.scalar.dma_start(hb_sb[:], hb_d.rearrange("(c p) d -> p c d", p=P))
            hb_bf = const_pool.tile([P, IC, D], bf16)
            nc.vector.tensor_copy(hb_bf[:], hb_sb[:])

            hout_ps = [
                hout_psum.tile([DT[t][1] - DT[t][0], R], f32, tag=f"ho{t}",
                               name=f"hout_ps{t}")
                for t in range(2)
            ]

            for w in range(NW):
                j0 = w * JW
                wlen = JW
                njc = JCW
                a_bf = []
                for ic in range(IC):
                    adj_t = adj_pool.tile([P, JW, 2], f32, tag=f"adj{ic}",
                                          name=f"adj_t{ic}")
                    nc.sync.dma_start(
                        adj_t[:],
                        adj_d[ic * P : (ic + 1) * P, j0 : j0 + JW, :],
                    )
                    ab = abf_pool.tile([P, JW], bf16, tag=f"abf{ic}",
                                       name=f"ab{ic}")
                    nc.vector.tensor_add(ab[:], adj_t[:, :, 0], adj_t[:, :, 1])
                    a_bf.append(ab)

                # PE-transpose the 16 [128,128] A tiles of this window,
                # 4 per PSUM tile; ScalarE evicts to SBUF
                at_tiles = []
                for jc in range(njc):
                    at_ps = at_psum.tile([P, R], bf16, tag="atps",
                                         name=f"at_ps{jc}")
                    for ic in range(IC):
                        nc.tensor.transpose(
                            at_ps[:, ic * P : (ic + 1) * P],
                            a_bf[ic][:, jc * P : (jc + 1) * P],
                            ident[:],
                        )
                    at_sb = at_pool.tile([P, R], bf16, tag=f"at{jc}",
                                         name=f"at_sb{jc}")
                    nc.scalar.copy(at_sb[:], at_ps[:])
                    at_tiles.append(at_sb)

                # p_inT[d, j] += h_blk[i, d] * A_blk[i, j], contract i
                for t, (d0, dn) in enumerate(DT):
                    pt = pin_psum.tile([dn - d0, JW], f32, tag=f"pt{t}",
                                       name=f"pt{t}")
                    for ic in range(IC):
                        nc.tensor.matmul(
                            pt[:],
                            hb_bf[:, ic, d0:dn],
                            a_bf[ic][:],
                            start=(ic == 0),
                            stop=(ic == IC - 1),
                        )
                    po = pout_pool.tile([dn - d0, JW], bf16, tag=f"po{t}",
                                        name=f"po{t}")
                    if w == NW - 1:
                        nc.vector.tensor_copy(po[:], pt[:])
                        nc.sync.dma_start(pin_d[d0:dn, j0 : j0 + JW], po[:])
                    else:
                        nc.scalar.copy(po[:], pt[:])
                        nc.scalar.dma_start(
                            pin_d[d0:dn, j0 : j0 + JW], po[:]
                        )

                # h_outT[d, i] += h[j, d] * A_blk[i, j], contract j
                for jc in range(njc):
                    g = w * JCW + jc
                    for t, (d0, dn) in enumerate(DT):
                        nc.tensor.matmul(
                            hout_ps[t][:],
                            h_bf[:, g, d0:dn],
                            at_tiles[jc][:],
                            start=(g == 0),
                            stop=(g == NJC - 1),
                        )

            for t, (d0, dn) in enumerate(DT):
                ho = pout_pool.tile([dn - d0, R], bf16, tag=f"hoev{t}",
                                    name=f"hoev{t}")
                nc.vector.tensor_copy(ho[:], hout_ps[t][:])
                nc.sync.dma_start(hout_d[d0:dn, :], ho[:])

    nc.compile()
    return nc


def _get_nc():
    if "nc" not in _NC_CACHE:
        _NC_CACHE["nc"] = _build_nc()
    return _NC_CACHE["nc"]


def _run_cores(adj, h, trace=False):
    from concourse.bass_utils import run_bass_kernel_spmd

    global LAST_RESULTS
    if trace:
        _ensure_ntff_hook()
    nc = _get_nc()
    in_maps = []
    for m in range(NCORES):
        in_maps.append(
            {
                "adj_blk": np.ascontiguousarray(adj[m * R : (m + 1) * R]),
                "h": h,
                "h_blk": np.ascontiguousarray(h[m * R : (m + 1) * R]),
            }
        )
    res = run_bass_kernel_spmd(
        nc, in_maps, core_ids=list(range(NCORES)), trace=trace
    )
    LAST_RESULTS = res
    return res


def kernel(unpreprocessed_unweight_adj_matrix, h):
    adj = np.ascontiguousarray(
        np.asarray(unpreprocessed_unweight_adj_matrix, dtype=np.float32)
    )
    h = np.ascontiguousarray(np.asarray(h, dtype=np.float32))
    res = _run_cores(adj, h)
    parts = res.results
    h_inT = np.zeros((D, L), dtype=np.float64)
    for r in parts:
        h_inT += np.asarray(r["p_inT"], dtype=np.float32).astype(np.float64)
    h_out = np.concatenate(
        [np.asarray(r["h_outT_blk"], dtype=np.float32).T for r in parts], axis=0
    )
    return (
        np.ascontiguousarray(h_inT.T).astype(np.float32),
        np.ascontiguousarray(h_out, dtype=np.float32),
    )



# revision 3
# speedup vs baseline: 1.0419x; 1.0419x over previous
"""Trainium2 Bass kernel for CalculateSLayer GNN message passing.

Computes, for adj [L, L, 2] f32 and h [L, D] f32 with A = adj.sum(-1):
    h_in[j, d]  = sum_i A[i, j] * h[i, d]   (= A.T @ h)
    h_out[i, d] = sum_j A[i, j] * h[j, d]   (= A @ h)

Sharding: rows of A across 8 NeuronCores. Core m holds A[m*512:(m+1)*512, :]:
  - h_out rows are fully local:      h_out_blk = A_blk @ h
  - h_in is a partial sum per core:  p_in      = A_blk.T @ h_blk
    (the 8 partials are summed on the host during unshard)

v2 layout (vs the first working version):
  - h / h_blk are pre-arranged ON THE HOST into the on-chip [p, chunk, d]
    bf16 layout and uploaded as such, so their DMAs are one fat contiguous
    descriptor per partition instead of 4096 x 600B descriptors (which
    previously stalled the ACT engine for ~15us and polluted the ring).
  - The sync HWDGE ring carries ONLY the adj stream: 2 x 1MB DMAs per
    512-wide j-window, every descriptor 4KB contiguous -> packet-rate
    limited at ~360 GB/s, i.e. the HBM roofline.
  - Half-window (256-row) granularity for edge-sum + PE work so compute
    trails the DMA stream closely (short pipeline tail).
  - PSUM: 8 banks exactly: 4 = p_inT double-buffered (2 d-tiles x 2),
    2 = h_outT accumulators, 2 = 4 half-bank A^T transpose tiles.
"""

import numpy as np

L = 4096
D = 150
NCORES = 8
R = L // NCORES  # 512 rows per core
P = 128  # partitions
IC = R // P  # 4 i-chunks per core
JW = 512  # j-window width
NW = L // JW  # 8 windows
NJC = L // P  # 32 j-chunks total

_NC_CACHE = {}
LAST_RESULTS = None


def _ensure_ntff_hook():
    """Register the axon NTFF profile hook if the image's antenv lacks it."""
    import sys
    import types

    try:
        from antenv.axon_hooks import get_axon_ntff_profile_hook  # noqa: F401

        return
    except ImportError:
        pass

    mod = types.ModuleType("antenv.axon_hooks")
    _state = {"hook": None}
    mod.set_axon_ntff_profile_hook = lambda h: _state.__setitem__("hook", h)
    mod.get_axon_ntff_profile_hook = lambda: _state["hook"]
    sys.modules["antenv.axon_hooks"] = mod
    import antenv

    antenv.axon_hooks = mod

    so_path = "/opt/axon/libaxon_pjrt.so"
    try:
        from trn_agent_boot.trn_boot import _ntff_profile_via_ctypes

        hook = _ntff_profile_via_ctypes(so_path)
        if hook is not None:
            mod.set_axon_ntff_profile_hook(hook)
    except Exception:
        pass

    try:
        from concourse import bass_utils

        bass_utils.upload_artifacts = lambda tmpdir: tmpdir
    except Exception:
        pass


def _build_nc():
    import concourse.bacc as bacc
    import concourse.tile as tile
    import concourse.mybir as mybir
    from concourse.masks import make_identity

    f32 = mybir.dt.float32
    bf16 = mybir.dt.bfloat16

    nc = bacc.Bacc(
        "TRN2", target_bir_lowering=False, debug=False, num_devices=NCORES
    )
    adj_d = nc.dram_tensor("adj_blk", [R, L, 2], f32, kind="ExternalInput").ap()
    # h pre-arranged on host: h_d[p, c, d] = h[c*128 + p, d], bf16
    h_d = nc.dram_tensor("h_pre", [P, NJC, D], bf16, kind="ExternalInput").ap()
    # this core's row block, hb_d[p, ic, d] = h[blk*512 + ic*128 + p, d]
    hb_d = nc.dram_tensor("hb_pre", [P, IC, D], bf16, kind="ExternalInput").ap()
    # outputs transposed: [D, ...]; host transposes back
    pin_d = nc.dram_tensor("p_inT", [D, L], bf16, kind="ExternalOutput").ap()
    hout_d = nc.dram_tensor("h_outT_blk", [D, R], bf16, kind="ExternalOutput").ap()

    DT = ((0, 128), (128, D))  # d-tile splits (M <= 128)

    # adj rows rearranged so partition p of half hf holds row hf*256+c*128+p
    adj_r = adj_d.rearrange("(c p) j e -> p c j e", p=P)  # [128, 4, L, 2]

    with tile.TileContext(nc) as tc:
        with (
            tc.tile_pool(name="const", bufs=1) as const_pool,
            tc.tile_pool(name="adj", bufs=3) as adj_pool,
            tc.tile_pool(name="abp", bufs=3) as ab_pool,
            tc.tile_pool(name="atp", bufs=2) as at_pool,
            tc.tile_pool(name="pouts", bufs=3) as pout_pool,
            tc.tile_pool(name="pinps", bufs=2, space="PSUM") as pin_psum,
            tc.tile_pool(name="atps", bufs=1, space="PSUM") as at_psum,
            tc.tile_pool(name="houtps", bufs=1, space="PSUM") as hout_psum,
        ):
            # ---- prologue ------------------------------------------------
            # adj window-0 DMAs are emitted first inside the loop below;
            # the sync ring carries nothing else.
            ident = const_pool.tile([P, P], bf16)

            hb_sb = const_pool.tile([P, IC, D], bf16)
            nc.scalar.dma_start(hb_sb[:], hb_d)
            h_sb = const_pool.tile([P, NJC, D], bf16)
            nc.scalar.dma_start(h_sb[:], h_d)

            make_identity(nc, ident[:])

            hout_ps = [
                hout_psum.tile([dn - d0, R], f32, tag=f"ho{t}",
                               name=f"hout_ps{t}")
                for t, (d0, dn) in enumerate(DT)
            ]

            for w in range(NW):
                j0 = w * JW

                # window-persistent tiles
                ab = ab_pool.tile([P, IC, JW], bf16, tag="ab", name="ab")
                # 2 jc's per 2KB PSUM bank: [128, 1024] bf16 = one bank
                at_pair = [
                    at_psum.tile([P, 2 * JW], bf16, tag=f"atps{pr}",
                                 name=f"at_pair{pr}")
                    for pr in range(2)
                ]
                at_ps = [
                    at_pair[jc // 2][:, (jc % 2) * JW : (jc % 2 + 1) * JW]
                    for jc in range(4)
                ]
                pt = [
                    pin_psum.tile([dn - d0, JW], f32, tag=f"pt{t}",
                                  name=f"pt{t}")
                    for t, (d0, dn) in enumerate(DT)
                ]
                at_sb = [
                    at_pool.tile([P, JW], bf16, tag=f"at{jc}",
                                 name=f"at_sb{jc}")
                    for jc in range(4)
                ]

                for hf in range(2):
                    ics = (2 * hf, 2 * hf + 1)
                    # 1MB DMA: rows hf*256..hf*256+255 of this window
                    adj_t = adj_pool.tile([P, 2, JW, 2], f32, tag=f"adj{hf}",
                                          name=f"adj_t{hf}")
                    nc.sync.dma_start(
                        adj_t[:],
                        adj_r[:, 2 * hf : 2 * hf + 2, j0 : j0 + JW, :],
                    )
                    # edge-channel sum -> bf16 A rows for this half
                    nc.vector.tensor_add(
                        ab[:, 2 * hf : 2 * hf + 2, :],
                        adj_t[:, :, :, 0],
                        adj_t[:, :, :, 1],
                    )

                    # p_inT[d, j] += h_blk[i, d] * A_blk[i, j]  (contract i)
                    for t, (d0, dn) in enumerate(DT):
                        for ic in ics:
                            nc.tensor.matmul(
                                pt[t][:],
                                hb_sb[:, ic, d0:dn],
                                ab[:, ic, :],
                                start=(ic == 0),
                                stop=(ic == IC - 1),
                            )
                    # PE-transpose this half's A tiles
                    for jc in range(4):
                        for ic in ics:
                            nc.tensor.transpose(
                                at_ps[jc][:, ic * P : (ic + 1) * P],
                                ab[:, ic, jc * P : (jc + 1) * P],
                                ident[:],
                            )
                        if hf == 1:
                            nc.scalar.copy(at_sb[jc][:], at_ps[jc][:])

                # h_outT[d, i] += h[j, d] * A_blk[i, j]  (contract j)
                for jc in range(4):
                    g = w * 4 + jc
                    for t, (d0, dn) in enumerate(DT):
                        nc.tensor.matmul(
                            hout_ps[t][:],
                            h_sb[:, g, d0:dn],
                            at_sb[jc][:],
                            start=(g == 0),
                            stop=(g == NJC - 1),
                        )

                # evict p_inT window and write out on the scalar ring
                for t, (d0, dn) in enumerate(DT):
                    po = pout_pool.tile([dn - d0, JW], bf16, tag=f"po{t}",
                                        name=f"po{t}")
                    nc.vector.tensor_copy(po[:], pt[t][:])
                    nc.scalar.dma_start(pin_d[d0:dn, j0 : j0 + JW], po[:])

            for t, (d0, dn) in enumerate(DT):
                ho = pout_pool.tile([dn - d0, R], bf16, tag=f"hoev{t}",
                                    name=f"hoev{t}")
                nc.vector.tensor_copy(ho[:], hout_ps[t][:])
                nc.scalar.dma_start(hout_d[d0:dn, :], ho[:])

    nc.compile()
    return nc


def _get_nc():
    if "nc" not in _NC_CACHE:
        _NC_CACHE["nc"] = _build_nc()
    return _NC_CACHE["nc"]


def _run_cores(adj, h, trace=False):
    import ml_dtypes
    from concourse.bass_utils import run_bass_kernel_spmd

    global LAST_RESULTS
    if trace:
        _ensure_ntff_hook()
    nc = _get_nc()
    bf16 = ml_dtypes.bfloat16
    # h_pre[p, c, d] = h[c*128 + p, d]
    h_pre = np.ascontiguousarray(
        h.reshape(NJC, P, D).transpose(1, 0, 2)
    ).astype(bf16)
    in_maps = []
    for m in range(NCORES):
        hb = h[m * R : (m + 1) * R].reshape(IC, P, D).transpose(1, 0, 2)
        in_maps.append(
            {
                "adj_blk": np.ascontiguousarray(adj[m * R : (m + 1) * R]),
                "h_pre": h_pre,
                "hb_pre": np.ascontiguousarray(hb).astype(bf16),
            }
        )
    res = run_bass_kernel_spmd(
        nc, in_maps, core_ids=list(range(NCORES)), trace=trace
    )
    LAST_RESULTS = res
    return res


def kernel(unpreprocessed_unweight_adj_matrix, h):
    adj = np.ascontiguousarray(
        np.asarray(unpreprocessed_unweight_adj_matrix, dtype=np.float32)
    )
    h = np.ascontiguousarray(np.asarray(h, dtype=np.float32))
    res = _run_cores(adj, h)
    parts = res.results
    h_inT = np.zeros((D, L), dtype=np.float64)
    for r in parts:
        h_inT += np.asarray(r["p_inT"], dtype=np.float32).astype(np.float64)
    h_out = np.concatenate(
        [np.asarray(r["h_outT_blk"], dtype=np.float32).T for r in parts], axis=0
    )
    return (
        np.ascontiguousarray(h_inT.T).astype(np.float32),
        np.ascontiguousarray(h_out, dtype=np.float32),
    )


# revision 4
# speedup vs baseline: 1.0493x; 1.0071x over previous
"""Trainium2 Bass kernel for CalculateSLayer GNN message passing.

Computes, for adj [L, L, 2] f32 and h [L, D] f32 with A = adj.sum(-1):
    h_in[j, d]  = sum_i A[i, j] * h[i, d]   (= A.T @ h)
    h_out[i, d] = sum_j A[i, j] * h[j, d]   (= A @ h)

Sharding: rows of A across 8 NeuronCores. Core m holds A[m*512:(m+1)*512, :]:
  - h_out rows are fully local:      h_out_blk = A_blk @ h
  - h_in is a partial sum per core:  p_in      = A_blk.T @ h_blk
    (the 8 partials are summed on the host during unshard)

v2 layout (vs the first working version):
  - h / h_blk are pre-arranged ON THE HOST into the on-chip [p, chunk, d]
    bf16 layout and uploaded as such, so their DMAs are one fat contiguous
    descriptor per partition instead of 4096 x 600B descriptors (which
    previously stalled the ACT engine for ~15us and polluted the ring).
  - The sync HWDGE ring carries ONLY the adj stream: 2 x 1MB DMAs per
    512-wide j-window, every descriptor 4KB contiguous -> packet-rate
    limited at ~360 GB/s, i.e. the HBM roofline.
  - Half-window (256-row) granularity for edge-sum + PE work so compute
    trails the DMA stream closely (short pipeline tail).
  - PSUM: 8 banks exactly: 4 = p_inT double-buffered (2 d-tiles x 2),
    2 = h_outT accumulators, 2 = 4 half-bank A^T transpose tiles.
"""

import numpy as np

L = 4096
D = 150
NCORES = 8
R = L // NCORES  # 512 rows per core
P = 128  # partitions
IC = R // P  # 4 i-chunks per core
JW = 512  # j-window width
NW = L // JW  # 8 windows
NJC = L // P  # 32 j-chunks total

_NC_CACHE = {}
LAST_RESULTS = None


def _ensure_ntff_hook():
    """Register the axon NTFF profile hook if the image's antenv lacks it."""
    import sys
    import types

    try:
        from antenv.axon_hooks import get_axon_ntff_profile_hook  # noqa: F401

        return
    except ImportError:
        pass

    mod = types.ModuleType("antenv.axon_hooks")
    _state = {"hook": None}
    mod.set_axon_ntff_profile_hook = lambda h: _state.__setitem__("hook", h)
    mod.get_axon_ntff_profile_hook = lambda: _state["hook"]
    sys.modules["antenv.axon_hooks"] = mod
    import antenv

    antenv.axon_hooks = mod

    so_path = "/opt/axon/libaxon_pjrt.so"
    try:
        from trn_agent_boot.trn_boot import _ntff_profile_via_ctypes

        hook = _ntff_profile_via_ctypes(so_path)
        if hook is not None:
            mod.set_axon_ntff_profile_hook(hook)
    except Exception:
        pass

    try:
        from concourse import bass_utils

        bass_utils.upload_artifacts = lambda tmpdir: tmpdir
    except Exception:
        pass


def _build_nc():
    import concourse.bacc as bacc
    import concourse.tile as tile
    import concourse.mybir as mybir
    from concourse.masks import make_identity

    f32 = mybir.dt.float32
    bf16 = mybir.dt.bfloat16

    nc = bacc.Bacc(
        "TRN2", target_bir_lowering=False, debug=False, num_devices=NCORES
    )
    adj_d = nc.dram_tensor("adj_blk", [R, L, 2], f32, kind="ExternalInput").ap()
    # h pre-arranged on host: h_d[p, c, d] = h[c*128 + p, d], bf16
    h_d = nc.dram_tensor("h_pre", [P, NJC, D], bf16, kind="ExternalInput").ap()
    # this core's row block, hb_d[p, ic, d] = h[blk*512 + ic*128 + p, d]
    hb_d = nc.dram_tensor("hb_pre", [P, IC, D], bf16, kind="ExternalInput").ap()
    # outputs transposed: [D, ...]; host transposes back
    pin_d = nc.dram_tensor("p_inT", [D, L], bf16, kind="ExternalOutput").ap()
    hout_d = nc.dram_tensor("h_outT_blk", [D, R], bf16, kind="ExternalOutput").ap()

    DT = ((0, 128), (128, D))  # d-tile splits (M <= 128)

    # adj rows rearranged so partition p of half hf holds row hf*256+c*128+p
    adj_r = adj_d.rearrange("(c p) j e -> p c j e", p=P)  # [128, 4, L, 2]

    with tile.TileContext(nc) as tc:
        with (
            tc.tile_pool(name="const", bufs=1) as const_pool,
            tc.tile_pool(name="adj", bufs=3) as adj_pool,
            tc.tile_pool(name="abp", bufs=3) as ab_pool,
            tc.tile_pool(name="atp", bufs=2) as at_pool,
            tc.tile_pool(name="pouts", bufs=3) as pout_pool,
            tc.tile_pool(name="pinps", bufs=2, space="PSUM") as pin_psum,
            tc.tile_pool(name="atps", bufs=1, space="PSUM") as at_psum,
            tc.tile_pool(name="houtps", bufs=1, space="PSUM") as hout_psum,
        ):
            # ---- prologue ------------------------------------------------
            # adj window-0 DMAs are emitted first inside the loop below;
            # the sync ring carries nothing else.
            ident = const_pool.tile([P, P], bf16)

            hb_sb = const_pool.tile([P, IC, D], bf16)
            nc.scalar.dma_start(hb_sb[:], hb_d)
            h_sb = const_pool.tile([P, NJC, D], bf16)
            nc.scalar.dma_start(h_sb[:], h_d)

            make_identity(nc, ident[:])

            hout_ps = [
                hout_psum.tile([dn - d0, R], f32, tag=f"ho{t}",
                               name=f"hout_ps{t}")
                for t, (d0, dn) in enumerate(DT)
            ]

            for w in range(NW):
                j0 = w * JW

                # window-persistent tiles
                ab = ab_pool.tile([P, IC, JW], bf16, tag="ab", name="ab")
                # 2 jc's per 2KB PSUM bank: [128, 1024] bf16 = one bank
                at_pair = [
                    at_psum.tile([P, 2 * JW], bf16, tag=f"atps{pr}",
                                 name=f"at_pair{pr}")
                    for pr in range(2)
                ]
                at_ps = [
                    at_pair[jc // 2][:, (jc % 2) * JW : (jc % 2 + 1) * JW]
                    for jc in range(4)
                ]
                pt = [
                    pin_psum.tile([dn - d0, JW], f32, tag=f"pt{t}",
                                  name=f"pt{t}")
                    for t, (d0, dn) in enumerate(DT)
                ]
                at_sb = [
                    at_pool.tile([P, JW], bf16, tag=f"at{jc}",
                                 name=f"at_sb{jc}")
                    for jc in range(4)
                ]

                for hf in range(2):
                    ics = (2 * hf, 2 * hf + 1)
                    # 1MB DMA: rows hf*256..hf*256+255 of this window
                    adj_t = adj_pool.tile([P, 2, JW, 2], f32, tag=f"adj{hf}",
                                          name=f"adj_t{hf}")
                    nc.sync.dma_start(
                        adj_t[:],
                        adj_r[:, 2 * hf : 2 * hf + 2, j0 : j0 + JW, :],
                    )
                    # edge-channel sum -> bf16 A rows, one op per i-chunk
                    for ic in ics:
                        nc.vector.tensor_add(
                            ab[:, ic, :],
                            adj_t[:, ic - 2 * hf, :, 0],
                            adj_t[:, ic - 2 * hf, :, 1],
                        )
                    # PE-transpose this half's A tiles; evict at half grain
                    # so the ACT engine is never behind the h_out matmuls
                    for jc in range(4):
                        for ic in ics:
                            nc.tensor.transpose(
                                at_ps[jc][:, ic * P : (ic + 1) * P],
                                ab[:, ic, jc * P : (jc + 1) * P],
                                ident[:],
                            )
                        nc.scalar.copy(
                            at_sb[jc][:, 2 * hf * P : (2 * hf + 2) * P],
                            at_ps[jc][:, 2 * hf * P : (2 * hf + 2) * P],
                        )

                # matmuls grouped by weight class (128-wide then 22-wide):
                # class switches cost ~100ns of exposed LDWEIGHTS each
                for t, (d0, dn) in enumerate(DT):
                    # p_inT[d, j] += h_blk[i, d] * A_blk[i, j]  (contract i)
                    for ic in range(IC):
                        nc.tensor.matmul(
                            pt[t][:],
                            hb_sb[:, ic, d0:dn],
                            ab[:, ic, :],
                            start=(ic == 0),
                            stop=(ic == IC - 1),
                        )
                    # h_outT[d, i] += h[j, d] * A_blk[i, j]  (contract j)
                    for jc in range(4):
                        g = w * 4 + jc
                        nc.tensor.matmul(
                            hout_ps[t][:],
                            h_sb[:, g, d0:dn],
                            at_sb[jc][:],
                            start=(g == 0),
                            stop=(g == NJC - 1),
                        )

                # evict p_inT window and write out on the scalar ring
                for t, (d0, dn) in enumerate(DT):
                    po = pout_pool.tile([dn - d0, JW], bf16, tag=f"po{t}",
                                        name=f"po{t}")
                    nc.vector.tensor_copy(po[:], pt[t][:])
                    nc.scalar.dma_start(pin_d[d0:dn, j0 : j0 + JW], po[:])

            for t, (d0, dn) in enumerate(DT):
                ho = pout_pool.tile([dn - d0, R], bf16, tag=f"hoev{t}",
                                    name=f"hoev{t}")
                nc.vector.tensor_copy(ho[:], hout_ps[t][:])
                nc.scalar.dma_start(hout_d[d0:dn, :], ho[:])

    nc.compile()
    return nc


def _get_nc():
    if "nc" not in _NC_CACHE:
        _NC_CACHE["nc"] = _build_nc()
    return _NC_CACHE["nc"]


def _run_cores(adj, h, trace=False):
    import ml_dtypes
    from concourse.bass_utils import run_bass_kernel_spmd

    global LAST_RESULTS
    if trace:
        _ensure_ntff_hook()
    nc = _get_nc()
    bf16 = ml_dtypes.bfloat16
    # h_pre[p, c, d] = h[c*128 + p, d]
    h_pre = np.ascontiguousarray(
        h.reshape(NJC, P, D).transpose(1, 0, 2)
    ).astype(bf16)
    in_maps = []
    for m in range(NCORES):
        hb = h[m * R : (m + 1) * R].reshape(IC, P, D).transpose(1, 0, 2)
        in_maps.append(
            {
                "adj_blk": np.ascontiguousarray(adj[m * R : (m + 1) * R]),
                "h_pre": h_pre,
                "hb_pre": np.ascontiguousarray(hb).astype(bf16),
            }
        )
    res = run_bass_kernel_spmd(
        nc, in_maps, core_ids=list(range(NCORES)), trace=trace
    )
    LAST_RESULTS = res
    return res


def kernel(unpreprocessed_unweight_adj_matrix, h):
    adj = np.ascontiguousarray(
        np.asarray(unpreprocessed_unweight_adj_matrix, dtype=np.float32)
    )
    h = np.ascontiguousarray(np.asarray(h, dtype=np.float32))
    res = _run_cores(adj, h)
    parts = res.results
    h_inT = np.zeros((D, L), dtype=np.float64)
    for r in parts:
        h_inT += np.asarray(r["p_inT"], dtype=np.float32).astype(np.float64)
    h_out = np.concatenate(
        [np.asarray(r["h_outT_blk"], dtype=np.float32).T for r in parts], axis=0
    )
    return (
        np.ascontiguousarray(h_inT.T).astype(np.float32),
        np.ascontiguousarray(h_out, dtype=np.float32),
    )


# revision 11
# speedup vs baseline: 1.0958x; 1.0444x over previous
"""Trainium2 Bass kernel for CalculateSLayer GNN message passing.

Computes, for adj [L, L, 2] f32 and h [L, D] f32 with A = adj.sum(-1):
    h_in[j, d]  = sum_i A[i, j] * h[i, d]   (= A.T @ h)
    h_out[i, d] = sum_j A[i, j] * h[j, d]   (= A @ h)

Sharding: rows of A across 8 NeuronCores. Core m holds A[m*512:(m+1)*512, :]:
  - h_out rows are fully local:      h_out_blk = A_blk @ h
  - h_in is a partial sum per core:  p_in      = A_blk.T @ h_blk
    (the 8 partials are summed on the host during unshard)

v2 layout (vs the first working version):
  - h / h_blk are pre-arranged ON THE HOST into the on-chip [p, chunk, d]
    bf16 layout and uploaded as such, so their DMAs are one fat contiguous
    descriptor per partition instead of 4096 x 600B descriptors (which
    previously stalled the ACT engine for ~15us and polluted the ring).
  - The sync HWDGE ring carries ONLY the adj stream: 2 x 1MB DMAs per
    512-wide j-window, every descriptor 4KB contiguous -> packet-rate
    limited at ~360 GB/s, i.e. the HBM roofline.
  - Half-window (256-row) granularity for edge-sum + PE work so compute
    trails the DMA stream closely (short pipeline tail).
  - PSUM: 8 banks exactly: 4 = p_inT double-buffered (2 d-tiles x 2),
    2 = h_outT accumulators, 2 = 4 half-bank A^T transpose tiles.
"""

import numpy as np

L = 4096
D = 150
NCORES = 8
R = L // NCORES  # 512 rows per core
P = 128  # partitions
IC = R // P  # 4 i-chunks per core
JW = 512  # j-window width
NW = L // JW  # 8 windows
NJC = L // P  # 32 j-chunks total

_NC_CACHE = {}
LAST_RESULTS = None


def _ensure_ntff_hook():
    """Register the axon NTFF profile hook if the image's antenv lacks it."""
    import sys
    import types

    try:
        from antenv.axon_hooks import get_axon_ntff_profile_hook  # noqa: F401

        return
    except ImportError:
        pass

    mod = types.ModuleType("antenv.axon_hooks")
    _state = {"hook": None}
    mod.set_axon_ntff_profile_hook = lambda h: _state.__setitem__("hook", h)
    mod.get_axon_ntff_profile_hook = lambda: _state["hook"]
    sys.modules["antenv.axon_hooks"] = mod
    import antenv

    antenv.axon_hooks = mod

    so_path = "/opt/axon/libaxon_pjrt.so"
    try:
        from trn_agent_boot.trn_boot import _ntff_profile_via_ctypes

        hook = _ntff_profile_via_ctypes(so_path)
        if hook is not None:
            mod.set_axon_ntff_profile_hook(hook)
    except Exception:
        pass

    try:
        from concourse import bass_utils

        bass_utils.upload_artifacts = lambda tmpdir: tmpdir
    except Exception:
        pass


def _build_nc():
    import concourse.bacc as bacc
    import concourse.tile as tile
    import concourse.mybir as mybir
    from concourse.masks import make_identity

    f32 = mybir.dt.float32
    bf16 = mybir.dt.bfloat16

    nc = bacc.Bacc(
        "TRN2", target_bir_lowering=False, debug=False, num_devices=NCORES
    )
    adj_d = nc.dram_tensor("adj_blk", [R, L, 2], f32, kind="ExternalInput").ap()
    # h pre-arranged on host: h_d[p, c, d] = h[c*128 + p, d], bf16
    h_d = nc.dram_tensor("h_pre", [P, NJC, D], bf16, kind="ExternalInput").ap()
    # this core's row block, hb_d[p, ic, d] = h[blk*512 + ic*128 + p, d]
    hb_d = nc.dram_tensor("hb_pre", [P, IC, D], bf16, kind="ExternalInput").ap()
    # outputs transposed: [D, ...]; host transposes back
    pin_d = nc.dram_tensor("p_inT", [D, L], bf16, kind="ExternalOutput").ap()
    hout_d = nc.dram_tensor("h_outT_blk", [D, R], bf16, kind="ExternalOutput").ap()

    # overlapping 128-wide d-tiles: identical 128x128 weight class for every
    # matmul (class switches cost ~100ns of exposed LDWEIGHTS); the d-rows
    # 22..127 of tile 1 are recomputed and discarded at eviction
    DT = ((0, 128), (D - 128, D))

    # adj rows rearranged so partition p of half hf holds row hf*256+c*128+p
    adj_r = adj_d.rearrange("(c p) j e -> p c j e", p=P)  # [128, 4, L, 2]

    with tile.TileContext(nc) as tc:
        with (
            tc.tile_pool(name="const", bufs=1) as const_pool,
            tc.tile_pool(name="adj", bufs=4) as adj_pool,
            tc.tile_pool(name="abp", bufs=4) as ab_pool,
            tc.tile_pool(name="atp", bufs=2) as at_pool,
            tc.tile_pool(name="pouts", bufs=3) as pout_pool,
            tc.tile_pool(name="pinps", bufs=2, space="PSUM") as pin_psum,
            tc.tile_pool(name="atps", bufs=1, space="PSUM") as at_psum,
            tc.tile_pool(name="houtps", bufs=1, space="PSUM") as hout_psum,
        ):
            # ---- prologue ------------------------------------------------
            # adj window-0 DMAs are emitted first inside the loop below;
            # the sync ring carries nothing else.
            ident = const_pool.tile([P, P], bf16)

            hb_sb = const_pool.tile([P, IC, D], bf16)
            nc.scalar.dma_start(hb_sb[:], hb_d)
            # 8 just-in-time pieces: window w's h_out matmuls only wait for
            # piece w (subtile deps), so the first windows aren't blocked
            # behind one big h transfer contending with the adj stream
            h_sb = const_pool.tile([P, NJC, D], bf16)
            for w in range(NW):
                nc.scalar.dma_start(
                    h_sb[:, 4 * w : 4 * w + 4, :], h_d[:, 4 * w : 4 * w + 4, :]
                )

            make_identity(nc, ident[:])

            hout_ps = [
                hout_psum.tile([P, R], f32, tag=f"ho{t}", name=f"hout_ps{t}")
                for t in range(2)
            ]

            for w in range(NW):
                j0 = w * JW

                # window-persistent tiles
                ab = ab_pool.tile([P, IC, JW], bf16, tag="ab", name="ab")
                # 2 jc's per 2KB PSUM bank: [128, 1024] bf16 = one bank
                at_pair = [
                    at_psum.tile([P, 2 * JW], bf16, tag=f"atps{pr}",
                                 name=f"at_pair{pr}")
                    for pr in range(2)
                ]
                at_ps = [
                    at_pair[jc // 2][:, (jc % 2) * JW : (jc % 2 + 1) * JW]
                    for jc in range(4)
                ]
                pt = [
                    pin_psum.tile([P, JW], f32, tag=f"pt{t}", name=f"pt{t}")
                    for t in range(2)
                ]
                # evicted as whole pairs (one ACT op per PSUM bank per
                # window): a per-jc eviction would make the second jc's
                # transposes wait on the first jc's eviction of the shared
                # bank, serializing the PE stream
                at_sbp = [
                    at_pool.tile([P, 2 * JW], bf16, tag=f"atp{pr}",
                                 name=f"at_sbp{pr}")
                    for pr in range(2)
                ]

                last = w == NW - 1
                for hf in range(2):
                    ics = (2 * hf, 2 * hf + 1)
                    if not last:
                        # 1MB DMA: rows hf*256..hf*256+255 of this window
                        adj_t = adj_pool.tile([P, 2, JW, 2], f32,
                                              tag=f"adj{hf}",
                                              name=f"adj_t{hf}")
                        nc.sync.dma_start(
                            adj_t[:],
                            adj_r[:, 2 * hf : 2 * hf + 2, j0 : j0 + JW, :],
                        )
                        parts = [(adj_t, 0, ics[0]), (adj_t, 1, ics[1])]
                    else:
                        # quarter-grain on the final window: compute can
                        # chase the last bytes chunk by chunk (short tail)
                        parts = []
                        for ic in ics:
                            adj_q = adj_pool.tile([P, 1, JW, 2], f32,
                                                  tag=f"adjq{ic}",
                                                  name=f"adj_q{ic}")
                            nc.sync.dma_start(
                                adj_q[:],
                                adj_r[:, ic : ic + 1, j0 : j0 + JW, :],
                            )
                            parts.append((adj_q, 0, ic))
                    # edge-channel sum -> bf16 A rows, one op per i-chunk
                    for tile_, sl, ic in parts:
                        nc.vector.tensor_add(
                            ab[:, ic, :],
                            tile_[:, sl, :, 0],
                            tile_[:, sl, :, 1],
                        )
                    # PE-transpose this half's A tiles
                    for jc in range(4):
                        for ic in ics:
                            nc.tensor.transpose(
                                at_ps[jc][:, ic * P : (ic + 1) * P],
                                ab[:, ic, jc * P : (jc + 1) * P],
                                ident[:],
                            )

                for pr in range(2):
                    nc.scalar.copy(at_sbp[pr][:], at_pair[pr][:])

                # p_inT[d, j] += h_blk[i, d] * A_blk[i, j]  (contract i);
                # pins first (ready as soon as the edge-sums land), h_outs
                # after (they additionally wait on the A^T evictions)
                for t, (d0, dn) in enumerate(DT):
                    for ic in range(IC):
                        nc.tensor.matmul(
                            pt[t][:],
                            hb_sb[:, ic, d0:dn],
                            ab[:, ic, :],
                            start=(ic == 0),
                            stop=(ic == IC - 1),
                        )
                # h_outT[d, i] += h[j, d] * A_blk[i, j]  (contract j)
                for t, (d0, dn) in enumerate(DT):
                    for jc in range(4):
                        g = w * 4 + jc
                        nc.tensor.matmul(
                            hout_ps[t][:],
                            h_sb[:, g, d0:dn],
                            at_sbp[jc // 2][:, (jc % 2) * JW : (jc % 2 + 1) * JW],
                            start=(g == 0),
                            stop=(g == NJC - 1),
                        )

                # evict p_inT window and write out on the scalar ring
                for t, (d0, dn) in enumerate(DT):
                    # PSUM reads need 32-aligned base partitions; slice the
                    # discarded overlap rows on the SBUF/DMA side instead
                    lo = 0 if t == 0 else 96
                    sk = 0 if t == 0 else 96 - (128 - (D - 128))
                    po = pout_pool.tile([128 - lo, JW], bf16, tag=f"po{t}",
                                        name=f"po{t}")
                    nc.vector.tensor_copy(po[:], pt[t][lo:128, :])
                    nc.scalar.dma_start(
                        pin_d[d0 + lo + sk : dn, j0 : j0 + JW],
                        po[sk - (96 - 96) if False else sk : 128 - lo, :],
                    )

            for t, (d0, dn) in enumerate(DT):
                lo = 0 if t == 0 else 96
                sk = 0 if t == 0 else 96 - (128 - (D - 128))
                ho = pout_pool.tile([128 - lo, R], bf16, tag=f"hoev{t}",
                                    name=f"hoev{t}")
                nc.vector.tensor_copy(ho[:], hout_ps[t][lo:128, :])
                nc.scalar.dma_start(
                    hout_d[d0 + lo + sk : dn, :], ho[sk : 128 - lo, :]
                )

    nc.compile()
    return nc


def _get_nc():
    if "nc" not in _NC_CACHE:
        _NC_CACHE["nc"] = _build_nc()
    return _NC_CACHE["nc"]


def _run_cores(adj, h, trace=False):
    import ml_dtypes
    from concourse.bass_utils import run_bass_kernel_spmd

    global LAST_RESULTS
    if trace:
        _ensure_ntff_hook()
    nc = _get_nc()
    bf16 = ml_dtypes.bfloat16
    # h_pre[p, c, d] = h[c*128 + p, d]
    h_pre = np.ascontiguousarray(
        h.reshape(NJC, P, D).transpose(1, 0, 2)
    ).astype(bf16)
    in_maps = []
    for m in range(NCORES):
        hb = h[m * R : (m + 1) * R].reshape(IC, P, D).transpose(1, 0, 2)
        in_maps.append(
            {
                "adj_blk": np.ascontiguousarray(adj[m * R : (m + 1) * R]),
                "h_pre": h_pre,
                "hb_pre": np.ascontiguousarray(hb).astype(bf16),
            }
        )
    res = run_bass_kernel_spmd(
        nc, in_maps, core_ids=list(range(NCORES)), trace=trace
    )
    LAST_RESULTS = res
    return res


def kernel(unpreprocessed_unweight_adj_matrix, h):
    adj = np.ascontiguousarray(
        np.asarray(unpreprocessed_unweight_adj_matrix, dtype=np.float32)
    )
    h = np.ascontiguousarray(np.asarray(h, dtype=np.float32))
    res = _run_cores(adj, h)
    parts = res.results
    h_inT = np.zeros((D, L), dtype=np.float64)
    for r in parts:
        h_inT += np.asarray(r["p_inT"], dtype=np.float32).astype(np.float64)
    h_out = np.concatenate(
        [np.asarray(r["h_outT_blk"], dtype=np.float32).T for r in parts], axis=0
    )
    return (
        np.ascontiguousarray(h_inT.T).astype(np.float32),
        np.ascontiguousarray(h_out, dtype=np.float32),
    )
